# revision 64
# baseline (speedup 1.0000x reference)
"""CASCADES adapter (moe_routing) on 8 TRN2 NeuronCores.

Reference computation (B=4, S=2048, D=4096, R=8, K=4):
    centroid[b] = 0.7*x[b,-1] + 0.3*mean_s x[b,s]
    w[b]        = softmax(cos(centroid[b], keys) / 0.05)
    Lam[b]      = sum_k w[b,k] * pool[k]                 # [R,R]
    out[b,s]    = gate * (x[b,s] @ V^T) @ Lam[b]^T @ U^T

Sharding: core i handles batch i//2, sequence half i%2 (1024 rows).
The only cross-core dependency is the full-sequence centroid; each core
contributes 0.3/2048*seqsum_local (+0.7*x_last on odd cores, passed as a
host-prepared aux input) and a pairwise exchange of [128,32] (16 KB)
produces the centroid on both cores of each pair.

Everything parameter-only is folded on the host: gate into U, the K
mixing matrices Mk = gate*U @ pool[k] (stacked as Mall^T [32,4096]), and
key normalization. On device the output matmul contracts over 32
partitions: lhsT = w-scaled 4x-replicated x_V^T tile [32,128], rhs =
Mall^T chunk [32,512].

v2 changes vs baseline:
  - x / V^T / Mall / identity are fed as float32r directly (raw fp32
    bits; the PE truncates internally) -> transposes run at 1.5
    cycles/row instead of 2.0 and all on-device fp32->fp32r cast
    copies disappear.
  - sequence partial sums come from one fused tensor_reduce per s-tile
    on the drained SBUF tile ([128,32,128] -> [128,32]) instead of 64
    per-PSUM-slice reduces: ~17us vector instead of ~40us.
  - PSUM drains alternate scalar/vector in both phases.
  - collective selectable (AllReduce / AllGather+add) for A/B timing.
"""

import os
import numpy as np
from contextlib import ExitStack

B, S, D, R, K = 4, 2048, 4096, 8, 4
NCORES = 8
SH = S // 2            # rows per core
PT = 128               # partition tile
NT = SH // PT          # 8 sequence tiles per core
NCH = D // PT          # 32 d-chunks
KR = K * R             # 32

CC_KIND = os.environ.get("CASC_CC", "ar")   # ar | ag | none

_CACHE = {}
LAST_RESULTS = None


def _build_program():
    from concourse import bacc, tile, mybir, library_config

    dt = mybir.dt.float32
    f32r = mybir.dt.float32r
    add = mybir.AluOpType.add
    mult = mybir.AluOpType.mult
    AF = mybir.ActivationFunctionType
    AX = mybir.AxisListType

    nc = bacc.Bacc("TRN2", target_bir_lowering=False, debug=False,
                   num_devices=NCORES)

    xs = nc.dram_tensor("xs", [SH, D], f32r, kind="ExternalInput").ap()
    vt = nc.dram_tensor("vt", [PT, NCH * KR], f32r, kind="ExternalInput").ap()
    mall = nc.dram_tensor("mall", [KR, D], f32r, kind="ExternalInput").ap()
    kcols = nc.dram_tensor("kcols", [PT, K * NCH], dt, kind="ExternalInput").ap()
    aux = nc.dram_tensor("aux", [PT, NCH], dt, kind="ExternalInput").ap()
    ident = nc.dram_tensor("ident", [PT, PT], f32r, kind="ExternalInput").ap()
    mask = nc.dram_tensor("mask", [KR, K], dt, kind="ExternalInput").ap()
    if CC_KIND == "ag8":
        gmask = nc.dram_tensor(
            "gmask", [PT, NCORES * NCH], dt, kind="ExternalInput").ap()
    out = nc.dram_tensor("out", [SH, D], dt, kind="ExternalOutput").ap()
    if CC_KIND == "probe":
        dbg = nc.dram_tensor("dbg", [PT, NCH], dt, kind="ExternalOutput").ap()

    NP_ = NT // 2      # s-tile pairs per core

    with tile.TileContext(nc) as tc, ExitStack() as c0:
        persist = c0.enter_context(tc.tile_pool(name="persist", bufs=1))
        dram = c0.enter_context(tc.tile_pool(name="dram", bufs=1, space="DRAM"))

        if os.environ.get("CASC_WARM", "1") == "1" and CC_KIND in ("ar", "ag8"):
            # size-matched dummy AllReduce emitted FIRST on the gpsimd
            # queue: it absorbs the CC-stack warmup concurrently with the
            # read phase, and triggering it as early as possible keeps it
            # clear of the real exchange even when the runtime's initial
            # CC barrier runs long (loaded host)
            wsb = persist.tile([PT, NCH], dt, name="wsb")
            nc.vector.memset(wsb[:], 1.0)
            din = dram.tile([PT, NCH], dt, name="din")
            nc.sync.dma_start(din[:], wsb[:])
            dout = dram.tile([PT, NCH], dt, name="dout")
            nc.gpsimd.collective_compute(
                "AllReduce",
                add,
                replica_groups=[[0, 1], [2, 3], [4, 5], [6, 7]],
                ins=[din.opt()],
                outs=[dout.opt()],
            )

        # ---- constants (gpsimd/SWDGE queue: don't block the x FIFO) ----
        kcols_sb = persist.tile([PT, K, NCH], dt, name="kcols_sb")
        nc.gpsimd.dma_start(kcols_sb[:], kcols[:])
        aux_sb = persist.tile([PT, NCH], dt, name="aux_sb")
        nc.gpsimd.dma_start(aux_sb[:], aux[:])
        ident_sb = persist.tile([PT, PT], f32r, name="ident_sb")
        nc.gpsimd.dma_start(ident_sb[:], ident[:])
        mask_sb = persist.tile([KR, K], dt, name="mask_sb")
        nc.gpsimd.dma_start(mask_sb[:], mask[:])
        ones_sb = persist.tile([PT, KR], dt, name="ones_sb")
        nc.vector.memset(ones_sb[:], 1.0)

        if CC_KIND in ("rd", "probe"):
            # pair exchange via direct SBUF->SBUF remote DMA (tpb XOR 1).
            # Allocation order is identical on every core (SPMD), so the
            # semaphore numbers agree across the pair.
            rsem = nc.alloc_semaphore("rd_rsem")
            lsem = nc.alloc_semaphore("rd_lsem")
            nc.gpsimd.sem_clear(rsem)
            nc.gpsimd.sem_clear(lsem)
            nc.gpsimd.load_library(library_config.remote_dma)
        vt_sb = persist.tile([PT, NCH, KR], f32r, name="vt_sb")
        nc.gpsimd.dma_start(vt_sb[:], vt[:].rearrange("p (c r) -> p c r", r=KR))
        mall_sb = persist.tile([KR, D], f32r, name="mall_sb")
        nc.gpsimd.dma_start(mall_sb[:], mall[:])

        # ---- persistent intermediates ----
        stash_sb = persist.tile([KR, NP_, 2 * PT], dt, name="stash_sb")
        # slot NT carries aux so the final reduce folds it in for free
        seqparts = persist.tile([PT, NCH, NT + 1], dt, name="seqparts")
        nc.vector.tensor_copy(seqparts[:, :, NT], aux_sb[:])
        cc_sb = persist.tile([PT, NCH], dt, name="cc_sb")
        cin = dram.tile([PT, NCH], dt, name="cin")

        # ================= read phase =================
        with ExitStack() as c1:
            xin = c1.enter_context(tc.tile_pool(name="xin", bufs=2))
            xtp = c1.enter_context(
                tc.tile_pool(name="xtp", bufs=6, space="PSUM"))
            xts = c1.enter_context(tc.tile_pool(name="xts", bufs=3))
            xvp = c1.enter_context(
                tc.tile_pool(name="xvp", bufs=2, space="PSUM"))

            def emit_xv(pr, xt_all):
                # x_V^T (4x-replicated rows) for both tiles of the pair:
                # out[kr, sub*128+s], contraction over d in fp32r
                xv_ps = xvp.tile([KR, 2 * PT], dt, name="xv_ps")
                for ch in range(NCH):
                    nc.tensor.matmul(
                        xv_ps[:], vt_sb[:, ch, :], xt_all[:, ch, :],
                        start=(ch == 0), stop=(ch == NCH - 1))
                nc.scalar.copy(stash_sb[:, pr, :], xv_ps[:])

            pend = []   # (pr, xt_all) with x_V not yet emitted
            for pr in range(NP_):
                # xt_all[p, ch, sub*128+s] = x[pair rows]^T, f32r, d-major
                xt_all = xts.tile([PT, NCH, 2 * PT], f32r, name="xt_all")
                for sub in range(2):
                    t = 2 * pr + sub
                    xtile = xin.tile([PT, D], f32r, name="xtile")
                    # split loads so early chunks transpose while the rest
                    # streams in; the final tile uses quarter granularity
                    # to shorten the post-stream routing-trigger chain
                    nld = 4 if t == NT - 1 else 2
                    q = D // nld
                    for li in range(nld):
                        nc.sync.dma_start(
                            xtile[:, li * q:(li + 1) * q],
                            xs[t * PT:(t + 1) * PT, li * q:(li + 1) * q])
                    for g in range(NCH // 4):
                        pt_ = xtp.tile([PT, 4, PT], f32r, name="pt_")
                        for j in range(4):
                            ch = 4 * g + j
                            nc.tensor.transpose(
                                pt_[:, j, :],
                                xtile[:, ch * PT:(ch + 1) * PT],
                                ident_sb[:],
                            )
                        # drains on scalar so vector only does the
                        # per-PSUM-slice sequence reduces; for the final
                        # tile alternate both engines to flush the
                        # end-of-stream backlog faster (the routing
                        # trigger waits on its last reduce)
                        dst = xt_all[:, 4 * g:4 * g + 4,
                                     sub * PT:(sub + 1) * PT]
                        if t == NT - 1 and g % 2 == 1:
                            nc.vector.tensor_copy(dst, pt_[:])
                        else:
                            nc.scalar.copy(dst, pt_[:])
                        nc.vector.tensor_reduce(
                            seqparts[:, 4 * g:4 * g + 4, t], pt_[:],
                            axis=AX.X, op=add)
                        # final centroid reduce in chunk-halves, emitted
                        # inline so the first half (+ its cin store)
                        # overlaps the last tile's remaining groups
                        if t == NT - 1 and CC_KIND == "ar":
                            if g == 3:
                                nc.vector.tensor_reduce(
                                    cc_sb[:, 0:16], seqparts[:, 0:16, :],
                                    axis=AX.X, op=add)
                                nc.sync.dma_start(
                                    cin[:, 0:16], cc_sb[:, 0:16])
                            elif g == 7:
                                nc.vector.tensor_reduce(
                                    cc_sb[:, 16:NCH],
                                    seqparts[:, 16:NCH, :],
                                    axis=AX.X, op=add)
                                nc.sync.dma_start(
                                    cin[:, 16:NCH], cc_sb[:, 16:NCH])

                # defer each pair's x_V matmuls so they never sit in the PE
                # queue ahead of a later tile's transposes (the routing
                # trigger only needs the transposes + reduces); emit just
                # late enough that the pool slot frees for pair pr+2
                pend.append((pr, xt_all))
                if pr >= 2:
                    emit_xv(*pend.pop(0))
            for args in pend:
                emit_xv(*args)

        # ================= routing =================
        # cosine similarity is scale-invariant, so the centroid's 0.3/S
        # factor is dropped; aux carries 0.7*S/0.3*x_last (folded into
        # seqparts slot NT, so the reduce includes it). For "ar" the
        # reduce + cin store were already emitted inline above.
        if CC_KIND != "ar":
            nc.vector.tensor_reduce(cc_sb[:], seqparts[:], axis=AX.X, op=add)

        c_sb = persist.tile([PT, NCH], dt, name="c_sb")
        if CC_KIND == "none":
            # timing-floor experiment: skip the exchange (wrong results)
            nc.vector.tensor_scalar_mul(c_sb[:], cc_sb[:], 2.0)
        elif CC_KIND == "probe":
            # no-wait remote-DMA probe (wrong results): send cc_sb to the
            # presumed pair partner, proceed without waiting, and dump
            # whatever landed in peer_sb at the very end of the kernel so
            # the host can identify the actual routing.
            peer_sb = persist.tile([PT, NCH], dt, name="peer_sb")
            nc.vector.memset(peer_sb[:], 0.0)
            nc.gpsimd.remote_dma_broadcast(
                peer_sb[:], cc_sb[:], rsem, lsem,
                rdests=[(0, 1)] + [None] * 7)
            nc.gpsimd.trigger_dma(None)
            nc.vector.tensor_scalar_mul(c_sb[:], cc_sb[:], 2.0)
        elif CC_KIND == "ag8":
            # single-group 8-core AllGather (shared output): measured floor
            # ~5us vs ~22us for 2-core-group collectives. Each core then
            # masks out its own pair's two slots and sums them.
            cin = dram.tile([PT, NCH], dt, name="cin")
            nc.sync.dma_start(cin[:], cc_sb[:])
            cout = dram.tile([NCORES, PT, NCH], dt, name="cout",
                             addr_space="Shared")
            nc.gpsimd.collective_compute(
                "AllGather",
                mybir.AluOpType.bypass,
                replica_groups=[list(range(NCORES))],
                ins=[cin.opt()],
                outs=[cout.opt()],
            )
            gath = persist.tile([PT, NCORES, NCH], dt, name="gath")
            nc.sync.dma_start(gath[:], cout[:].rearrange("t p c -> p t c"))
            gm_sb = persist.tile([PT, NCORES, NCH], dt, name="gm_sb")
            nc.gpsimd.dma_start(
                gm_sb[:], gmask[:].rearrange("p (t c) -> p t c", c=NCH))
            gt = persist.tile([PT, NCORES, NCH], dt, name="gt")
            nc.vector.tensor_mul(gt[:], gath[:], gm_sb[:])
            g4 = persist.tile([PT, 4, NCH], dt, name="g4")
            nc.vector.tensor_add(g4[:], gt[:, 0:4, :], gt[:, 4:8, :])
            g2 = persist.tile([PT, 2, NCH], dt, name="g2")
            nc.vector.tensor_add(g2[:], g4[:, 0:2, :], g4[:, 2:4, :])
            nc.vector.tensor_add(c_sb[:], g2[:, 0, :], g2[:, 1, :])
        elif CC_KIND == "rd":
            # SBUF->SBUF remote DMA to tpb^1; Q7 relative routing means the
            # same SPMD program sends to each core's pair partner. The
            # receive-side semaphore wait is attached AFTER TileContext
            # scheduling (the single-core scheduling simulator cannot model
            # the peer's increment and would report a deadlock).
            peer_sb = persist.tile([PT, NCH], dt, name="peer_sb")
            nc.gpsimd.remote_dma_broadcast(
                peer_sb[:], cc_sb[:], rsem, lsem,
                rdests=[(0, 1)] + [None] * 7)
            nc.gpsimd.trigger_dma(None)
            rd_add_bi = nc.vector.tensor_add(c_sb[:], cc_sb[:], peer_sb[:])
        else:
            if CC_KIND != "ar":
                # ar's cin was stored inline in chunk-halves above
                cin = dram.tile([PT, NCH], dt, name="cin")
                nc.sync.dma_start(cin[:], cc_sb[:])
            if CC_KIND == "ar":
                cout = dram.tile([PT, NCH], dt, name="cout")
                nc.gpsimd.collective_compute(
                    "AllReduce",
                    add,
                    replica_groups=[[0, 1], [2, 3], [4, 5], [6, 7]],
                    ins=[cin.opt()],
                    outs=[cout.opt()],
                )
                nc.sync.dma_start(c_sb[:], cout[:])
            else:  # ag
                cout = dram.tile([2, PT, NCH], dt, name="cout")
                nc.gpsimd.collective_compute(
                    "AllGather",
                    mybir.AluOpType.bypass,
                    replica_groups=[[0, 1], [2, 3], [4, 5], [6, 7]],
                    ins=[cin.opt()],
                    outs=[cout.opt()],
                )
                peer_sb = persist.tile([PT, 2, NCH], dt, name="peer_sb")
                nc.sync.dma_start(
                    peer_sb[:], cout[:].rearrange("t p c -> p t c"))
                nc.vector.tensor_add(
                    c_sb[:], peer_sb[:, 0, :], peer_sb[:, 1, :])

        # per-partition partial dots: <c,kn_k> (k=0..3) and |c|^2.
        # replicate c 4x (two doubling copies) so all four key dots run as
        # one multiply + one reduce instead of four of each.
        partials = persist.tile([PT, K + 1], dt, name="partials")
        c4 = persist.tile([PT, K, NCH], dt, name="c4")
        nc.vector.tensor_copy(c4[:, 0, :], c_sb[:])
        nc.vector.tensor_copy(c4[:, 1, :], c4[:, 0, :])
        nc.vector.tensor_copy(c4[:, 2:4, :], c4[:, 0:2, :])
        junk4 = persist.tile([PT, K, NCH], dt, name="junk4")
        nc.vector.tensor_mul(junk4[:], c4[:], kcols_sb[:])
        nc.vector.tensor_reduce(
            partials[:, 0:K], junk4[:], axis=AX.X, op=add)
        junk = persist.tile([PT, NCH], dt, name="junk")
        nc.vector.tensor_mul(junk[:], c_sb[:], c_sb[:])
        nc.vector.tensor_reduce(
            partials[:, K:K + 1], junk[:], axis=AX.X, op=add)

        with ExitStack() as cm, \
                tc.tile_pool(name="rps", bufs=1, space="PSUM") as rps:
            del cm
            r_ps = rps.tile([KR, K + 1], dt, name="r_ps")
            nc.tensor.matmul(r_ps[:], ones_sb[:], partials[:],
                             start=True, stop=True)
            rt_sb = persist.tile([KR, K + 1], dt, name="rt_sb")
            nc.scalar.copy(rt_sb[:], r_ps[:])

        # cn = sqrt(|c|^2 * T^2) = T*|c|  (T = 0.05 folded into the sqrt
        # scale), so exp(sim/T) = exp(<c,kn> * rcn) with rcn = 1/(T*|c|)
        cn = persist.tile([KR, 1], dt, name="cn")
        nc.scalar.activation(cn[:], rt_sb[:, K:K + 1], AF.Sqrt,
                             0.0, 0.05 * 0.05)
        rcn = persist.tile([KR, 1], dt, name="rcn")
        nc.vector.reciprocal(rcn[:], cn[:])
        ex = persist.tile([KR, K], dt, name="ex")
        nc.scalar.activation(ex[:], rt_sb[:, 0:K], AF.Exp, 0.0, rcn[:])
        ssum = persist.tile([KR, 1], dt, name="ssum")
        nc.vector.tensor_reduce(ssum[:], ex[:], axis=AX.X, op=add)
        rsum = persist.tile([KR, 1], dt, name="rsum")
        nc.vector.reciprocal(rsum[:], ssum[:])
        wmat = persist.tile([KR, K], dt, name="wmat")
        nc.vector.tensor_scalar_mul(wmat[:], ex[:], rsum[:])
        wcol = persist.tile([KR, 1], dt, name="wcol")
        junk2 = persist.tile([KR, K], dt, name="junk2")
        nc.vector.tensor_mul(junk2[:], wmat[:], mask_sb[:])
        nc.vector.tensor_reduce(wcol[:], junk2[:], axis=AX.X, op=add)

        # ================= write phase =================
        with ExitStack() as c2:
            otp = c2.enter_context(
                tc.tile_pool(name="otp", bufs=6, space="PSUM"))
            osb_pool = c2.enter_context(tc.tile_pool(name="osb", bufs=3))
            xvw_pool = c2.enter_context(tc.tile_pool(name="xvw", bufs=2))

            for t in range(NT):
                xvw = xvw_pool.tile([KR, PT], f32r, name="xvw")
                src = stash_sb[:, t // 2, (t % 2) * PT:(t % 2 + 1) * PT]
                if t == 0:
                    # vector produced wcol: same-engine issue avoids a
                    # cross-engine hop on the very first output tile
                    nc.vector.tensor_scalar_mul(xvw[:], src, wcol[:])
                else:
                    nc.scalar.mul(xvw[:], src, wcol[:])
                osb = osb_pool.tile([PT, D], dt, name="osb")
                for n in range(D // 512):
                    o_ps = otp.tile([PT, 512], dt, name="o_ps")
                    nc.tensor.matmul(
                        o_ps[:], xvw[:], mall_sb[:, n * 512:(n + 1) * 512],
                        start=True, stop=True)
                    dst = osb[:, n * 512:(n + 1) * 512]
                    if n % 2 == 0:
                        nc.scalar.copy(dst, o_ps[:])
                    else:
                        nc.vector.tensor_copy(dst, o_ps[:])
                # finer stores split across two HWDGE queues so the write
                # stream starts as soon as the first chunks drain; tile 0
                # streams at chunk granularity to hide the post-routing
                # pipeline fill
                # all on the sync queue: the scalar/vector instruction
                # streams are busy draining PSUM, so issuing stores from
                # them delays the store behind queued drains
                nds = 8 if t == 0 else 4
                q = D // nds
                for i in range(nds):
                    nc.sync.dma_start(
                        out[t * PT:(t + 1) * PT, i * q:(i + 1) * q],
                        osb[:, i * q:(i + 1) * q])

        if CC_KIND == "probe":
            # dump whatever the peer delivered (~100us after the send, so
            # arrival is not in doubt if the routing works at all)
            nc.sync.dma_start(dbg[:], peer_sb[:])

    if CC_KIND == "rd":
        # The single-core scheduling simulator can't model the peer's
        # semaphore increment (it would report a deadlock), so the receive
        # wait is spliced in AFTER Tile scheduling: a standalone DVE
        # EventSemaphore wait (rsem >= 2 = 16//8 engine lanes) placed
        # immediately before the pair-sum in its basic block.
        fn = nc.m.functions[0]
        w = nc.vector.wait_ge(rsem, 2)
        for b in fn.blocks:
            if any(i.name == w.ins.name for i in b.instructions):
                b.instructions.remove(w.ins)
                break
        for b in fn.blocks:
            names = [i.name for i in b.instructions]
            if rd_add_bi.ins.name in names:
                b.instructions.insert(names.index(rd_add_bi.ins.name), w.ins)
                break
        else:
            raise RuntimeError("rd: pair-sum instruction not found")

    nc.compile()
    return nc


def _get_program():
    if "nc" not in _CACHE:
        _CACHE["nc"] = _build_program()
    return _CACHE["nc"]


def _host_prep(x, U, V, pool, keys, gate_w, gate_b):
    """Parameter-only folding + per-core shard/aux construction."""
    f32 = np.float32
    # gate (parameter-only)
    gin = np.concatenate([U.mean(axis=0), V.mean(axis=1)]).astype(f32)
    z = gin @ gate_w[0].astype(f32) + gate_b[0].astype(f32)
    gate = f32(1.0) / (f32(1.0) + np.exp(-z, dtype=f32))
    Ug = (gate * U).astype(f32)

    # Mall^T [32, 4096]: rows 8k+j = (gate*U @ pool[k])[:, j]
    mall = np.concatenate(
        [(Ug @ pool[k]).T.astype(f32) for k in range(K)], axis=0)
    mall = np.ascontiguousarray(mall, dtype=f32)

    # V^T in column-chunk layout, replicated 4x along r:
    # [p, c*KR + k*R + r] = V[r, c*128+p]
    vt = np.ascontiguousarray(
        np.tile(V.T.reshape(NCH, PT, R), (1, 1, K))
        .transpose(1, 0, 2).reshape(PT, NCH * KR),
        dtype=f32)

    # normalized keys in column layout [128, K*32]: [p, k*32+c] = kn[k, c*128+p]
    knorm = np.maximum(np.linalg.norm(keys, axis=1, keepdims=True), 1e-8)
    kn = (keys / knorm).astype(f32)
    kcols = np.ascontiguousarray(
        kn.reshape(K, NCH, PT).transpose(2, 0, 1).reshape(PT, K * NCH),
        dtype=f32)

    identity = np.eye(PT, dtype=f32)
    msk = np.zeros((KR, K), dtype=f32)
    for p in range(KR):
        msk[p, p // R] = 1.0

    shared = {"vt": vt, "mall": mall, "kcols": kcols, "ident": identity,
              "mask": msk}

    in_maps = []
    for core in range(NCORES):
        b, h = divmod(core, 2)
        xsrd = np.ascontiguousarray(x[b, h * SH:(h + 1) * SH, :], dtype=f32)
        if h == 1:
            # routing centroid is used only through cosine similarity, so
            # scale by S/0.3 to fold away the on-device 0.3/S factor
            aux = np.ascontiguousarray(
                (f32(0.7 * S / 0.3) * x[b, S - 1, :]).reshape(NCH, PT).T,
                dtype=f32)
        else:
            aux = np.zeros((PT, NCH), dtype=f32)
        m = {"xs": xsrd, "aux": aux, **shared}
        if CC_KIND == "ag8":
            gm = np.zeros((PT, NCORES, NCH), dtype=f32)
            gm[:, 2 * b, :] = 1.0
            gm[:, 2 * b + 1, :] = 1.0
            m["gmask"] = gm.reshape(PT, NCORES * NCH)
        in_maps.append(m)
    return in_maps


def kernel(x, U_shared, V_shared, core_pool, core_keys, gate_w, gate_b):
    global LAST_RESULTS
    from concourse import bass_utils

    x = np.asarray(x, dtype=np.float32)
    U = np.asarray(U_shared, dtype=np.float32)
    V = np.asarray(V_shared, dtype=np.float32)
    pool = np.asarray(core_pool, dtype=np.float32)
    keys = np.asarray(core_keys, dtype=np.float32)
    gw = np.asarray(gate_w, dtype=np.float32)
    gb = np.asarray(gate_b, dtype=np.float32)

    nc = _get_program()
    in_maps = _host_prep(x, U, V, pool, keys, gw, gb)
    res = bass_utils.run_bass_kernel_spmd(
        nc, in_maps, core_ids=list(range(NCORES)))
    LAST_RESULTS = res

    out = np.empty((B, S, D), dtype=np.float32)
    for core in range(NCORES):
        b, h = divmod(core, 2)
        out[b, h * SH:(h + 1) * SH, :] = res.results[core]["out"]
    return out
